# revision 1
# baseline (speedup 1.0000x reference)
"""Trainium2 Bass kernel for nn_CannyEdge (16,3,512,512) -> (16,3,512,512).

Math (verified bit-equivalent to the reference on the fixed input):
  - All 3 output channels are identical; out = f(sum over channels of blurred img).
  - Separable gaussian (reflect pad) + separable sobel (zero pad).
  - NMS decisions taken in msq = gx^2+gy^2 domain (sqrt is monotone).
  - is_max_k = [sum over batch+sides of (msq > shifted msq) == 32] (PE-summed).
  - orientation class from signs of p = gx*gy and D = gy^2 - gx^2.
  - out = mag * is_max_cls.

Sharding: spatial row-strips. Core k owns image rows [64k, 64k+64) of ALL 16
samples (the batch-global min of the reference stays core-local). Rows split
into 2 bands of 32; each (sample, band) is a 40-row strip (32 + 4 halo);
strips pack 3-per-tile on partition slots 0/40/80; 6 tiles per band.
Vertical taps/shifts are banded-matrix matmuls on PE (engines are
partition-lockstep); horizontal shifts are free-dim APs on DVE.
"""

import os

import numpy as np

import concourse.bacc as bacc
import concourse.mybir as mybir
from concourse.mybir import AluOpType as Op
from concourse.tile import TileContext
from concourse.bass_utils import run_bass_kernel_spmd

F32 = mybir.dt.float32
AF = mybir.ActivationFunctionType

B, C, H, W = 16, 3, 512, 512
NCORES = 8
ROWS = H // NCORES          # 64 output rows per core
SH = 32                     # block output rows
HALO = 4
SIN = SH + 2 * HALO         # 40 strip rows
PACK = 3
NBANDS = 2
TPB = 6                     # tiles per band: 5 full + 1 single
NT = NBANDS * TPB
NP = PACK * SIN             # 120 partitions used
BFREE = TPB * W             # 3072
FREE = NT * W               # 6144
NVAR = 4                    # matrix variants: (band h) x (full | single)

KSIZE, SIGMA = 5, 1.4
PAIRS = [(0, 1), (-1, 1), (-1, 0), (-1, -1)]  # E, NE, N, NW
SKIP_PHASEB = bool(os.environ.get("CANNY_SKIP_PHASEB"))
NPAIRS = int(os.environ.get("CANNY_NPAIRS", "4"))
SKIP_STAGE_DEFAULT = int(os.environ.get("CANNY_STAGE", "4"))
USE_POOL = bool(os.environ.get("CANNY_USE_POOL"))

MATNAMES = ("vb", "vs", "vsn", "dv", "dv2", "sel", "shu", "shd")


def _gauss1d():
    half = (KSIZE - 1) * 0.5
    x = np.linspace(-half, half, KSIZE, dtype=np.float32)
    pdf = np.exp(np.float32(-0.5) * (x / np.float32(SIGMA)) ** 2).astype(np.float32)
    return (pdf / pdf.sum()).astype(np.float32)


def _slots(t):
    return [3 * t + j for j in range(PACK) if 3 * t + j < B]


def _band_lhsT(core, h, nslots, taps, offs, mode, out_lo, out_hi):
    """lhsT (K=NP, M=NP) for a vertical conv: out[m] = sum_k lhsT[k,m] x[k]."""
    M = np.zeros((NP, NP), np.float32)
    gr0 = ROWS * core + SH * h - HALO
    for j in range(nslots):
        for io in range(out_lo, out_hi + 1):
            if not (0 <= gr0 + io < H):
                continue  # out row outside image -> column stays 0
            for tap, d in zip(taps, offs):
                g = gr0 + io + d
                if 0 <= g < H:
                    isrc = io + d
                elif mode == "reflect":
                    g2 = -g if g < 0 else 2 * (H - 1) - g
                    isrc = g2 - gr0
                else:
                    continue
                M[SIN * j + isrc, SIN * j + io] += tap
    return M


def _sel3(nslots):
    """V accumulate+replicate: out[40j'+i] += sum_j bits[40j+i], i in 4..35."""
    M = np.zeros((NP, NP), np.float32)
    for j in range(nslots):
        for jp in range(PACK):
            for i in range(HALO, HALO + SH):
                M[SIN * j + i, SIN * jp + i] = 1.0
    return M


def _build_core_inputs(img, core):
    g = _gauss1d()
    k0, k1, k2 = float(g[0]), float(g[1]), float(g[2])

    chin = np.zeros((C, NP, FREE), np.float32)
    for h in range(NBANDS):
        gr0 = ROWS * core + SH * h - HALO
        for t in range(TPB):
            T = TPB * h + t
            for j, s in enumerate(_slots(t)):
                lo = max(0, gr0)
                hi = min(H, gr0 + SIN)
                chin[:, SIN * j + (lo - gr0):SIN * j + (hi - gr0),
                     T * W:(T + 1) * W] = img[s, :, lo:hi, :]

    # matrix variants: v = 2*h + (1 if single-strip tile else 0)
    mats = {n: np.zeros((NVAR, NP, NP), np.float32) for n in MATNAMES}
    for h in range(NBANDS):
        for single in (0, 1):
            v = 2 * h + single
            ns = 1 if single else PACK
            mats["vb"][v] = _band_lhsT(core, h, ns, [k0, k1, k2, k1, k0],
                                       [-2, -1, 0, 1, 2], "reflect", 2, SIN - 3)
            mats["vs"][v] = _band_lhsT(core, h, ns, [1.0, 2.0, 1.0],
                                       [-1, 0, 1], "zero", 3, SIN - 4)
            mats["dv"][v] = _band_lhsT(core, h, ns, [1.0, -1.0],
                                       [-1, 1], "zero", 3, SIN - 4)
            mats["vsn"][v] = -mats["vs"][v]
            mats["dv2"][v] = 2.0 * mats["dv"][v]
            mats["shu"][v] = _band_lhsT(core, h, ns, [1.0], [-1], "zero",
                                        HALO, HALO + SH - 1)
            mats["shd"][v] = _band_lhsT(core, h, ns, [1.0], [1], "zero",
                                        HALO, HALO + SH - 1)
            mats["sel"][v] = _sel3(ns)

    def tr(a):  # (NVAR,NP,NP) -> (NP, NVAR*NP)
        return np.ascontiguousarray(a.transpose(1, 0, 2).reshape(NP, NVAR * NP))

    out = {n: tr(mats[n]) for n in MATNAMES}
    out["chin"] = chin
    return out


def _build_bass(reps=1, stage=None, npairs=None, skipb=None):
    STAGE = SKIP_STAGE_DEFAULT if stage is None else stage
    NPAIRS_ = NPAIRS if npairs is None else npairs
    SKIPB = SKIP_PHASEB if skipb is None else skipb
    g = _gauss1d()
    r0 = float(g[0] / g[1])           # k0/k1
    r1 = float(g[1] / g[2])           # k1/k2
    sc = float(g[2]) * float(g[2])    # k2^2 folded into sqrt

    nc = bacc.Bacc("TRN2", target_bir_lowering=False, debug=False,
                   num_devices=NCORES)

    chin = nc.dram_tensor("chin", [C, NP, FREE], F32, kind="ExternalInput").ap()
    dmats = {n: nc.dram_tensor(n, [NP, NVAR * NP], F32,
                               kind="ExternalInput").ap() for n in MATNAMES}
    outp = nc.dram_tensor("outp", [NP, FREE], F32, kind="ExternalOutput").ap()

    with TileContext(nc) as tc:
        with (
            tc.tile_pool(name="const", bufs=1) as cpool,
            tc.tile_pool(name="chp", bufs=1) as chpool,
            tc.tile_pool(name="persist", bufs=1) as ppool,
            tc.tile_pool(name="work", bufs=2) as wpool,
            tc.tile_pool(name="bits", bufs=1) as bpool,
            tc.tile_pool(name="pb", bufs=1) as pbpool,
            tc.tile_pool(name="ptv", bufs=1, space="PSUM") as ptv,
            tc.tile_pool(name="pgx", bufs=1, space="PSUM") as pgx,
            tc.tile_pool(name="pgy", bufs=1, space="PSUM") as pgy,
            tc.tile_pool(name="pv", bufs=1, space="PSUM") as pvpool,
        ):
            smats = {}
            for name in MATNAMES:
                mt = cpool.tile([NP, NVAR * NP], F32, tag=name, name=f"m_{name}")
                nc.sync.dma_start(out=mt[:], in_=dmats[name])
                smats[name] = mt

            def mat(name, h, t):
                v = 2 * h + (1 if len(_slots(t)) == 1 else 0)
                return smats[name][:, v * NP:(v + 1) * NP]

            msq_s = ppool.tile([NP, BFREE], F32, tag="msq")
            p_s = ppool.tile([NP, BFREE], F32, tag="p")
            d_s = ppool.tile([NP, BFREE], F32, tag="d")
            out_s = ppool.tile([NP, BFREE], F32, tag="out")
            xpl = ppool.tile([NP, 4 * W], F32, tag="xpl")

            def xk(k):
                return xpl[:, k * W:(k + 1) * W]

            for rep in range(reps):
                for h in range(NBANDS):
                    ch_s = chpool.tile([NP, C * BFREE], F32, tag="ch",
                                       name=f"ch{rep}_{h}")
                    for c in range(C):
                        nc.sync.dma_start(
                            out=ch_s[:, c * BFREE:(c + 1) * BFREE],
                            in_=chin[c, :, h * BFREE:(h + 1) * BFREE])

                    vps = [pvpool.tile([NP, W], F32, tag=f"v{k}", name=f"vps{rep}_{h}{k}")
                           for k in range(4)]

                    for t in range(TPB):
                        fs = slice(t * W, (t + 1) * W)

                        def chs(c):
                            return ch_s[:, c * BFREE + t * W:c * BFREE + (t + 1) * W]

                        # vertical gaussian + channel sum (PE)
                        tv = ptv.tile([NP, W], F32, tag="tv", name=f"tv{rep}_{h}{t}")
                        for c in range(C):
                            nc.tensor.matmul(tv[:], mat("vb", h, t), chs(c),
                                             start=(c == 0), stop=(c == C - 1))
                        tvs = wpool.tile([NP, W], F32, tag="tvs", name=f"tvs{rep}_{h}{t}")
                        nc.scalar.activation(tvs[:], tv[:], AF.Copy)

                        # horizontal gaussian (DVE), scaled by 1/k2, reflect pad
                        u1 = wpool.tile([NP, W], F32, bufs=1, tag="u1", name=f"u1{rep}_{h}{t}")
                        nc.vector.tensor_tensor(u1[:, 2:510], tvs[:, 0:508],
                                                tvs[:, 4:512], Op.add)
                        nc.vector.tensor_scalar_mul(u1[:, 0:1], tvs[:, 2:3], 2.0)
                        nc.vector.tensor_tensor(u1[:, 1:2], tvs[:, 1:2],
                                                tvs[:, 3:4], Op.add)
                        nc.vector.tensor_tensor(u1[:, 510:511], tvs[:, 508:509],
                                                tvs[:, 510:511], Op.add)
                        nc.vector.tensor_scalar_mul(u1[:, 511:512],
                                                    tvs[:, 509:510], 2.0)
                        u2 = wpool.tile([NP, W], F32, bufs=1, tag="u2", name=f"u2{rep}_{h}{t}")
                        nc.vector.tensor_tensor(u2[:, 1:511], tvs[:, 0:510],
                                                tvs[:, 2:512], Op.add)
                        nc.vector.tensor_scalar_mul(u2[:, 0:1], tvs[:, 1:2], 2.0)
                        nc.vector.tensor_scalar_mul(u2[:, 511:512],
                                                    tvs[:, 510:511], 2.0)
                        vv = wpool.tile([NP, W], F32, bufs=1, tag="vv", name=f"vv{rep}_{h}{t}")
                        nc.vector.scalar_tensor_tensor(vv[:], u1[:], r0, u2[:],
                                                       Op.mult, Op.add)
                        tt = wpool.tile([NP, W], F32, tag="tt", name=f"tt{rep}_{h}{t}")
                        nc.vector.scalar_tensor_tensor(tt[:], vv[:], r1, tvs[:],
                                                       Op.mult, Op.add)

                        if STAGE < 2:
                            nc.vector.tensor_copy(msq_s[:, fs], tt[:])
                            continue
                        # sobel: horizontal +-1 shifts folded into PE via
                        # column-ranged matmuls (edge cols get zero-pad free)
                        gx = pgx.tile([NP, W], F32, tag="gx", name=f"gx{rep}_{h}{t}")
                        nc.tensor.matmul(gx[:, 1:512], mat("vs", h, t),
                                         tt[:, 0:511], start=True, stop=False)
                        nc.tensor.matmul(gx[:, 0:511], mat("vsn", h, t),
                                         tt[:, 1:512], start=False, stop=True)
                        gy = pgy.tile([NP, W], F32, tag="gy", name=f"gy{rep}_{h}{t}")
                        nc.tensor.matmul(gy[:, 1:512], mat("dv", h, t),
                                         tt[:, 0:511], start=True, stop=False)
                        nc.tensor.matmul(gy[:, 0:511], mat("dv", h, t),
                                         tt[:, 1:512], start=False, stop=False)
                        nc.tensor.matmul(gy[:], mat("dv2", h, t), tt[:],
                                         start=False, stop=True)

                        # squares, msq, p, D
                        sqx = wpool.tile([NP, W], F32, bufs=1, tag="sqx", name=f"sqx{rep}_{h}{t}")
                        nc.scalar.activation(sqx[:], gx[:], AF.Square)
                        sqy = wpool.tile([NP, W], F32, bufs=1, tag="sqy", name=f"sqy{rep}_{h}{t}")
                        nc.scalar.activation(sqy[:], gy[:], AF.Square)
                        gxs = wpool.tile([NP, W], F32, bufs=1, tag="gxs", name=f"gxs{rep}_{h}{t}")
                        nc.scalar.activation(gxs[:], gx[:], AF.Copy)
                        (nc.gpsimd if USE_POOL else nc.vector).tensor_tensor(
                            msq_s[:, fs], sqx[:], sqy[:], Op.add)
                        nc.vector.tensor_tensor(p_s[:, fs], gxs[:], gy[:], Op.mult)
                        (nc.gpsimd if USE_POOL else nc.vector).tensor_tensor(
                            d_s[:, fs], sqy[:], sqx[:], Op.subtract)

                        if STAGE < 3:
                            continue
                        # vertical +-1 shifts of msq (PE; reuses gx/gy PSUM slots)
                        m = msq_s[:, fs]
                        mu = pgx.tile([NP, W], F32, tag="gx", name=f"mu{rep}_{h}{t}")
                        nc.tensor.matmul(mu[:], mat("shu", h, t), m,
                                         start=True, stop=True)   # mu[i]=m[i-1]
                        md = pgy.tile([NP, W], F32, tag="gy", name=f"md{rep}_{h}{t}")
                        nc.tensor.matmul(md[:], mat("shd", h, t), m,
                                         start=True, stop=True)   # md[i]=m[i+1]

                        def vsrc(dr):
                            return m if dr == 0 else (mu if dr == -1 else md)

                        # one-sided pass bits; V accumulation (PE), target sum 32
                        for k, (dr, dc) in enumerate(PAIRS[:NPAIRS_]):
                            for sgn in (1, -1):
                                rdr, rdc = dr * sgn, dc * sgn
                                s = vsrc(rdr)
                                eng = nc.vector
                                bt = bpool.tile([NP, W], F32, tag=f"b{k}{sgn}",
                                                name=f"bt{rep}_{h}{t}{k}{sgn}")
                                lo, hi = max(0, -rdc), W - max(0, rdc)
                                eng.tensor_tensor(
                                    bt[:, lo:hi], s[:, lo + rdc:hi + rdc],
                                    m[:, lo:hi], Op.is_lt)
                                if rdc > 0:
                                    eng.tensor_scalar(
                                        bt[:, W - 1:W], m[:, W - 1:W], 0.0, None,
                                        Op.is_gt)
                                elif rdc < 0:
                                    eng.tensor_scalar(
                                        bt[:, 0:1], m[:, 0:1], 0.0, None, Op.is_gt)
                                nc.tensor.matmul(
                                    vps[k][:], mat("sel", h, t), bt[:],
                                    start=(t == 0 and sgn == 1),
                                    stop=(t == TPB - 1 and sgn == -1))

                    # x planes for this band: all 32 one-sided tests passed
                    for k in (range(NPAIRS_) if STAGE >= 3 else []):
                        nc.vector.tensor_scalar(xk(k), vps[k][:], 32.0, None,
                                                Op.is_equal)

                    # phase B: gate + magnitude (band-level ops)
                    if SKIPB or STAGE < 4:
                        nc.scalar.activation(out_s[:], msq_s[:], AF.Sqrt,
                                             scale=sc)
                        nc.sync.dma_start(
                            out=outp[:, h * BFREE:(h + 1) * BFREE],
                            in_=out_s[:])
                        continue
                    import dataclasses as _dc

                    def rep6(apx):
                        return _dc.replace(apx, ap=[apx.ap[0], [0, TPB],
                                                    apx.ap[1]])

                    def as3(apx):
                        return apx.rearrange("p (s w) -> p s w", w=W)

                    nc.scalar.activation(out_s[:], msq_s[:], AF.Sqrt, scale=sc)
                    vsel = pbpool.tile([NP, BFREE], mybir.dt.uint8, tag="vsel",
                                      name=f"vsel{rep}_{h}")
                    nc.vector.tensor_scalar(vsel[:], p_s[:], 0.0, None,
                                            Op.is_lt)
                    asel = pbpool.tile([NP, BFREE], mybir.dt.uint8, tag="asel",
                                      name=f"asel{rep}_{h}")
                    nc.vector.tensor_scalar(asel[:], d_s[:], 0.0, None,
                                            Op.is_ge)
                    yp = pbpool.tile([NP, BFREE], F32, tag="yp",
                                    name=f"yp{rep}_{h}")
                    nc.vector.tensor_copy(as3(yp[:]), rep6(xk(0)))
                    nc.vector.copy_predicated(as3(yp[:]), as3(asel[:]),
                                              rep6(xk(1)))
                    yn = pbpool.tile([NP, BFREE], F32, tag="yn",
                                    name=f"yn{rep}_{h}")
                    nc.vector.tensor_copy(as3(yn[:]), rep6(xk(3)))
                    nc.vector.copy_predicated(as3(yn[:]), as3(asel[:]),
                                              rep6(xk(2)))
                    nc.vector.copy_predicated(yp[:], vsel[:], yn[:])
                    nc.vector.tensor_tensor(out_s[:], out_s[:], yp[:], Op.mult)

                    nc.sync.dma_start(out=outp[:, h * BFREE:(h + 1) * BFREE],
                                      in_=out_s[:])

    nc.compile()
    return nc


_NC_CACHE = None


def kernel(img):
    global _NC_CACHE
    img = np.ascontiguousarray(np.asarray(img, dtype=np.float32))
    assert img.shape == (B, C, H, W)

    if _NC_CACHE is None:
        _NC_CACHE = _build_bass()
    nc = _NC_CACHE

    in_maps = [_build_core_inputs(img, core) for core in range(NCORES)]
    trace = bool(os.environ.get("CANNY_TRACE"))
    res = run_bass_kernel_spmd(nc, in_maps, core_ids=list(range(NCORES)),
                               trace=trace)
    if trace and res.exec_time_ns is not None:
        print(f"HW exec time: {res.exec_time_ns} ns")
        kernel.last_exec_ns = res.exec_time_ns

    out = np.zeros((B, C, H, W), np.float32)
    for core in range(NCORES):
        o = res.results[core]["outp"]
        for h in range(NBANDS):
            r0b = ROWS * core + SH * h
            for t in range(TPB):
                T = TPB * h + t
                for j, s in enumerate(_slots(t)):
                    blk = o[SIN * j + HALO:SIN * j + HALO + SH,
                            T * W:(T + 1) * W]
                    out[s, :, r0b:r0b + SH, :] = blk[None]
    return out


if __name__ == "__main__":
    img = np.load("/tmp/img.npy")
    out = kernel(img)
    exp = np.load("/tmp/expected.npy")
    d = np.abs(out - exp)
    print("absmax", d.max(), "n>1e-2", (d > 1e-2).sum(),
          "keepmis", ((out != 0) != (exp != 0)).sum())



# revision 12
# speedup vs baseline: 11.1668x; 11.1668x over previous
"""Trainium2 Bass kernel for nn_CannyEdge (16,3,512,512) -> (16,3,512,512).

v3: full-fp16 pipeline (validated offline on the fixed input: rel ~1.1e-3,
zero keep-mask flips vs the fp32 reference; tolerance is 2e-2).

Math (same as v2, bit-faithful NMS decisions at fp16 precision):
  - All 3 output channels identical; out = f(sum over channels of img).
  - blur2d fused on PE: 10 banded-matmul accumulations (5 col-shifted taps
    + 5 single-col reflect corrections) directly from the channel-summed
    input -> tt (PSUM).
  - sobel gx/gy via column-ranged banded matmuls on tt (fp16).
  - NMS in msq = gx^2+gy^2 domain; per-pair one-sided tests folded via
    bmax = max(side+, side-); bit = (bmax < msq). Batch-AND via fp16 sel
    matmuls accumulating bit sums in PSUM; is_max = (sum == 16).
  - orientation from signs of p = gx*gy and d = sqy-sqx.
  - keep-mask selected arithmetically (exact 0/1 fp16), out = mag * keep.

Sharding: spatial row-strips. Core k owns image rows [64k, 64k+64) of ALL
16 samples (batch-global min stays core-local). 2 bands x 6 tiles of
[120, 512]; strips are 40 rows (32 + 2*4 halo), 3 samples per tile.
"""

import os

import numpy as np

import concourse.bacc as bacc
import concourse.mybir as mybir
from concourse.mybir import AluOpType as Op
from concourse.tile import TileContext
from concourse.bass_utils import run_bass_kernel_spmd

F32 = mybir.dt.float32
F16 = mybir.dt.float16
AF = mybir.ActivationFunctionType

B, C, H, W = 16, 3, 512, 512
NCORES = 8
ROWS = H // NCORES          # 64 output rows per core
SH = 32                     # block output rows
HALO = 4
SIN = SH + 2 * HALO         # 40 strip rows
PACK = 3
NBANDS = 2
TPB = 6                     # tiles per band: 5 full + 1 single
NP = PACK * SIN             # 120 partitions used
BFREE = TPB * W             # 3072
FREE = NBANDS * BFREE       # 6144
NVAR = 4                    # matrix variants: (band h) x (full | single)

KSIZE, SIGMA = 5, 1.4
# pair order ni=0..3: E, NE, N, NW (reference PAIRS)
# bit-plane order in the packed bits4/bmax4 buffer: (NE, EW, NS, NW) is NOT
# used; we use (NE, EW, NW, NS) so edge fixes hit planes 0,1,2 with stride.
PLANE_OF_PAIR = {0: 1, 1: 0, 2: 3, 3: 2}   # pair ni -> plane index

MATNAMES = ("vb0", "vb1", "vb2", "vs", "vsn", "dv", "dv2", "shu", "shd",
            "sel")


def _gauss1d():
    half = (KSIZE - 1) * 0.5
    x = np.linspace(-half, half, KSIZE, dtype=np.float32)
    pdf = np.exp(np.float32(-0.5) * (x / np.float32(SIGMA)) ** 2).astype(np.float32)
    return (pdf / pdf.sum()).astype(np.float32)


def _slots(t):
    return [3 * t + j for j in range(PACK) if 3 * t + j < B]


def _band_lhsT(core, h, nslots, taps, offs, mode, out_lo, out_hi):
    """lhsT (K=NP, M=NP) for a vertical conv: out[m] = sum_k lhsT[k,m] x[k]."""
    M = np.zeros((NP, NP), np.float32)
    gr0 = ROWS * core + SH * h - HALO
    for j in range(nslots):
        for io in range(out_lo, out_hi + 1):
            if not (0 <= gr0 + io < H):
                continue
            for tap, d in zip(taps, offs):
                g = gr0 + io + d
                if 0 <= g < H:
                    isrc = io + d
                elif mode == "reflect":
                    g2 = -g if g < 0 else 2 * (H - 1) - g
                    isrc = g2 - gr0
                else:
                    continue
                M[SIN * j + isrc, SIN * j + io] += tap
    return M


def _sel3(nslots):
    """V accumulate+replicate: out[40j'+i] += sum_j bits[40j+i], i in 4..35."""
    M = np.zeros((NP, NP), np.float32)
    for j in range(nslots):
        for jp in range(PACK):
            for i in range(HALO, HALO + SH):
                M[SIN * j + i, SIN * jp + i] = 1.0
    return M


def _build_core_inputs(img, core):
    # channel-presummed fp16 input
    S = img.sum(axis=1, dtype=np.float32)          # (B, H, W)
    chin = np.zeros((NP, FREE), np.float16)
    for h in range(NBANDS):
        gr0 = ROWS * core + SH * h - HALO
        for t in range(TPB):
            T = TPB * h + t
            for j, s in enumerate(_slots(t)):
                lo = max(0, gr0)
                hi = min(H, gr0 + SIN)
                chin[SIN * j + (lo - gr0):SIN * j + (hi - gr0),
                     T * W:(T + 1) * W] = S[s, lo:hi, :]

    g = _gauss1d()
    k0, k1, k2 = float(g[0]), float(g[1]), float(g[2])

    mats = {n: np.zeros((NVAR, NP, NP), np.float32) for n in MATNAMES}
    for h in range(NBANDS):
        for single in (0, 1):
            v = 2 * h + single
            ns = 1 if single else PACK
            vb = _band_lhsT(core, h, ns, [k0, k1, k2, k1, k0],
                            [-2, -1, 0, 1, 2], "reflect", 2, SIN - 3)
            mats["vb0"][v] = k0 * vb
            mats["vb1"][v] = k1 * vb
            mats["vb2"][v] = k2 * vb
            mats["vs"][v] = _band_lhsT(core, h, ns, [1.0, 2.0, 1.0],
                                       [-1, 0, 1], "zero", 3, SIN - 4)
            mats["vsn"][v] = -mats["vs"][v]
            mats["dv"][v] = _band_lhsT(core, h, ns, [1.0, -1.0],
                                       [-1, 1], "zero", 3, SIN - 4)
            mats["dv2"][v] = 2.0 * mats["dv"][v]
            mats["shu"][v] = _band_lhsT(core, h, ns, [1.0], [-1], "zero",
                                        HALO, HALO + SH - 1)
            mats["shd"][v] = _band_lhsT(core, h, ns, [1.0], [1], "zero",
                                        HALO, HALO + SH - 1)
            mats["sel"][v] = _sel3(ns)

    def tr(a):  # (NVAR,NP,NP) -> (NP, NVAR*NP) fp16
        return np.ascontiguousarray(
            a.transpose(1, 0, 2).reshape(NP, NVAR * NP)).astype(np.float16)

    out = {n: tr(mats[n]) for n in MATNAMES}
    out["chin"] = chin
    return out


def _build_bass(reps=1):
    g = _gauss1d()
    k2 = float(g[2])
    sc = float(k2 * k2)          # tt is gauss/k2; msq = tt-based -> scale
    # mag = sqrt(msq_true) = sqrt(msq_tt * k2^4)? NO:
    # tt = blur / k2  (both v and h taps folded? v-tap full, h-tap /k2)
    # Here blur2d fused with true v taps and h taps {k0,k1,k2}: tt = blur
    # exactly, so mag = sqrt(msq). sc = 1.
    sc = 1.0

    nc = bacc.Bacc("TRN2", target_bir_lowering=False, debug=False,
                   num_devices=NCORES)

    chin = nc.dram_tensor("chin", [NP, FREE], F16, kind="ExternalInput").ap()
    dmats = {n: nc.dram_tensor(n, [NP, NVAR * NP], F16,
                               kind="ExternalInput").ap() for n in MATNAMES}
    outp = nc.dram_tensor("outp", [NP, FREE], F16, kind="ExternalOutput").ap()

    with TileContext(nc) as tc:
        with (
            tc.tile_pool(name="const", bufs=1) as cpool,
            tc.tile_pool(name="chp", bufs=1) as chpool,
            tc.tile_pool(name="band", bufs=1) as bpool,
            tc.tile_pool(name="work", bufs=2) as wpool,
            tc.tile_pool(name="ptt", bufs=1, space="PSUM") as ptt,
            tc.tile_pool(name="pgx", bufs=1, space="PSUM") as pgx,
            tc.tile_pool(name="pgy", bufs=1, space="PSUM") as pgy,
            tc.tile_pool(name="pv", bufs=1, space="PSUM") as pvpool,
        ):
            smats = {}
            for name in MATNAMES:
                mt = cpool.tile([NP, NVAR * NP], F16, tag=name, name=f"m_{name}")
                nc.sync.dma_start(out=mt[:], in_=dmats[name])
                smats[name] = mt

            def mat(name, h, t):
                v = 2 * h + (1 if len(_slots(t)) == 1 else 0)
                return smats[name][:, v * NP:(v + 1) * NP]

            # persistent band buffers (fp16):
            # smm = [muh | msqh | mdh] contiguous, each [NP, BFREE]
            smm = bpool.tile([NP, 3 * BFREE], F16, tag="smm", name="smm")
            sqx_b = bpool.tile([NP, BFREE], F16, tag="sqx", name="sqx_b")
            sqy_b = bpool.tile([NP, BFREE], F16, tag="sqy", name="sqy_b")
            pt_b = bpool.tile([NP, BFREE], F16, tag="ptb", name="pt_b")
            asel_b = bpool.tile([NP, BFREE], F16, tag="aselb", name="asel_b")
            vsel_b = bpool.tile([NP, BFREE], F16, tag="vselb", name="vsel_b")
            bm4 = bpool.tile([NP, 4 * BFREE], F16, tag="bm4", name="bm4")
            bits4 = bpool.tile([NP, 4 * BFREE], F16, tag="bits4", name="bits4")
            xpl = bpool.tile([NP, 4 * W], F16, tag="xpl", name="xpl")
            ed = bpool.tile([NP, 4 * W], F16, tag="ed", name="ed")
            keep_b = bpool.tile([NP, BFREE], F16, tag="keepb", name="keep_b")
            t2_b = bpool.tile([NP, BFREE], F16, tag="t2b", name="t2_b")
            mag_b = bpool.tile([NP, BFREE], F16, tag="magb", name="mag_b")
            out_b = bpool.tile([NP, BFREE], F16, tag="outb", name="out_b")

            def muh(sl=slice(None)):
                return smm[:, 0 * BFREE:1 * BFREE][:, sl]

            def msqh(sl=slice(None)):
                return smm[:, 1 * BFREE:2 * BFREE][:, sl]

            def mdh(sl=slice(None)):
                return smm[:, 2 * BFREE:3 * BFREE][:, sl]

            def plane(buf, k, sl=slice(None)):
                return buf[:, k * BFREE:(k + 1) * BFREE][:, sl]

            def xk(k):
                return xpl[:, k * W:(k + 1) * W]

            def xrep(apx):   # [NP, W] -> broadcast over TPB tiles
                import dataclasses as _dc
                return _dc.replace(apx, ap=[apx.ap[0], [0, TPB], apx.ap[1]])

            for rep in range(reps):
                for h in range(NBANDS):
                    ch_s = chpool.tile([NP, BFREE], F16, tag="ch",
                                       name=f"ch{rep}_{h}")
                    nc.sync.dma_start(out=ch_s[:],
                                      in_=chin[:, h * BFREE:(h + 1) * BFREE])

                    vps = [pvpool.tile([NP, W], F32, tag=f"v{k}",
                                       name=f"vps{rep}_{h}{k}")
                           for k in range(4)]

                    for t in range(TPB):
                        fs = slice(t * W, (t + 1) * W)
                        chs = ch_s[:, fs]

                        # fused blur2d on PE: tt = sum_j kj * VB @ ch[c+j]
                        tt = ptt.tile([NP, W], F32, tag="tt", name=f"tt{rep}_{h}{t}")
                        # base taps, col-ranged; j = -2..2 uses vb(|j|)
                        nc.tensor.matmul(tt[:, 0:510], mat("vb0", h, t),
                                         chs[:, 2:512], start=True, stop=False)
                        nc.tensor.matmul(tt[:, 2:512], mat("vb0", h, t),
                                         chs[:, 0:510], start=False, stop=False)
                        nc.tensor.matmul(tt[:, 0:511], mat("vb1", h, t),
                                         chs[:, 1:512], start=False, stop=False)
                        nc.tensor.matmul(tt[:, 1:512], mat("vb1", h, t),
                                         chs[:, 0:511], start=False, stop=False)
                        nc.tensor.matmul(tt[:, 0:512], mat("vb2", h, t),
                                         chs[:, 0:512], start=False, stop=False)
                        # reflect corrections (left: cols 0,1; right: 510,511)
                        nc.tensor.matmul(tt[:, 0:1], mat("vb1", h, t),
                                         chs[:, 1:2], start=False, stop=False)
                        nc.tensor.matmul(tt[:, 0:1], mat("vb0", h, t),
                                         chs[:, 2:3], start=False, stop=False)
                        nc.tensor.matmul(tt[:, 1:2], mat("vb0", h, t),
                                         chs[:, 1:2], start=False, stop=False)
                        nc.tensor.matmul(tt[:, 511:512], mat("vb1", h, t),
                                         chs[:, 510:511], start=False, stop=False)
                        nc.tensor.matmul(tt[:, 511:512], mat("vb0", h, t),
                                         chs[:, 509:510], start=False, stop=False)
                        nc.tensor.matmul(tt[:, 510:511], mat("vb0", h, t),
                                         chs[:, 510:511], start=False, stop=True)

                        tth = wpool.tile([NP, W], F16, tag="tth",
                                         name=f"tth{rep}_{h}{t}")
                        nc.scalar.activation(tth[:], tt[:], AF.Copy)

                        # sobel on PE (fp16), zero pad at image edges
                        gx = pgx.tile([NP, W], F32, tag="gx", name=f"gx{rep}_{h}{t}")
                        nc.tensor.matmul(gx[:, 1:512], mat("vs", h, t),
                                         tth[:, 0:511], start=True, stop=False)
                        nc.tensor.matmul(gx[:, 0:511], mat("vsn", h, t),
                                         tth[:, 1:512], start=False, stop=True)
                        gy = pgy.tile([NP, W], F32, tag="gy", name=f"gy{rep}_{h}{t}")
                        nc.tensor.matmul(gy[:, 1:512], mat("dv", h, t),
                                         tth[:, 0:511], start=True, stop=False)
                        nc.tensor.matmul(gy[:, 0:511], mat("dv", h, t),
                                         tth[:, 1:512], start=False, stop=False)
                        nc.tensor.matmul(gy[:], mat("dv2", h, t), tth[:],
                                         start=False, stop=True)

                        nc.scalar.activation(sqx_b[:, fs], gx[:], AF.Square)
                        nc.scalar.activation(sqy_b[:, fs], gy[:], AF.Square)
                        gxh = wpool.tile([NP, W], F16, tag="gxh",
                                         name=f"gxh{rep}_{h}{t}")
                        nc.scalar.activation(gxh[:], gx[:], AF.Copy)
                        nc.vector.tensor_tensor(pt_b[:, fs], gxh[:], gy[:],
                                                Op.mult)

                    # band: msq
                    nc.vector.tensor_tensor(msqh(), sqx_b[:], sqy_b[:], Op.add)

                    # per tile: vertical shifts of msq on PE -> muh/mdh
                    for t in range(TPB):
                        fs = slice(t * W, (t + 1) * W)
                        mu = pgx.tile([NP, W], F32, tag="gx", name=f"mu{rep}_{h}{t}")
                        nc.tensor.matmul(mu[:], mat("shu", h, t), msqh(fs),
                                         start=True, stop=True)
                        nc.scalar.activation(muh(fs), mu[:], AF.Copy)
                        md = pgy.tile([NP, W], F32, tag="gy", name=f"md{rep}_{h}{t}")
                        nc.tensor.matmul(md[:], mat("shd", h, t), msqh(fs),
                                         start=True, stop=True)
                        nc.scalar.activation(mdh(fs), md[:], AF.Copy)

                    # band: orientation masks
                    nc.vector.tensor_tensor(asel_b[:], sqy_b[:], sqx_b[:],
                                            Op.is_ge)
                    nc.vector.tensor_scalar(vsel_b[:], pt_b[:], 0.0, None,
                                            Op.is_lt)

                    # band: bmax planes (order NE, EW, NW, NS)
                    # NE: max(muh[c+1], mdh[c-1]); junction cols fixed later
                    nc.vector.tensor_tensor(plane(bm4, 0, slice(1, BFREE - 1)),
                                            muh(slice(2, BFREE)),
                                            mdh(slice(0, BFREE - 2)), Op.max)
                    # EW: max(msqh[c+1], msqh[c-1])
                    nc.vector.tensor_tensor(plane(bm4, 1, slice(1, BFREE - 1)),
                                            msqh(slice(2, BFREE)),
                                            msqh(slice(0, BFREE - 2)), Op.max)
                    # NW: max(muh[c-1], mdh[c+1])
                    nc.vector.tensor_tensor(plane(bm4, 2, slice(1, BFREE - 1)),
                                            muh(slice(0, BFREE - 2)),
                                            mdh(slice(2, BFREE)), Op.max)
                    # NS: max(muh, mdh) - no column shift, full width
                    nc.vector.tensor_tensor(plane(bm4, 3), muh(), mdh(),
                                            Op.max)

                    # band: all 4 bit planes in one op (rhs = msqh broadcast)
                    import dataclasses as _dc
                    m4 = _dc.replace(msqh(), ap=[msqh().ap[0], [0, 4],
                                                 msqh().ap[1]])
                    b4v = bits4[:].rearrange("p (k f) -> p k f", k=4)
                    bm4v = bm4[:].rearrange("p (k f) -> p k f", k=4)
                    nc.vector.tensor_tensor(b4v, bm4v, m4, Op.is_lt)

                    # per-tile edge fixes: one TT [NP,3] per image edge col
                    for t in range(TPB):
                        c0 = t * W
                        c1 = t * W + (W - 1)
                        # col 0: planes (NE,EW,NW) <- lhs (muh,msqh,mdh)[c0+1]
                        a_o0 = bits4[:, c0:c0 + 1]
                        o0 = _dc.replace(a_o0, ap=[a_o0.ap[0], [BFREE, 3], [1, 1]])
                        a_l0 = smm[:, c0 + 1:c0 + 2]
                        l0 = _dc.replace(a_l0, ap=[a_l0.ap[0], [BFREE, 3], [1, 1]])
                        a_r0 = msqh(slice(c0, c0 + 1))
                        r0 = _dc.replace(a_r0, ap=[a_r0.ap[0], [0, 3], [1, 1]])
                        nc.vector.tensor_tensor(o0, l0, r0, Op.is_lt)
                        # col 511: planes (NW,EW,NE) <- lhs (muh,msqh,mdh)[c1-1]
                        a_o1 = bits4[:, 2 * BFREE + c1:2 * BFREE + c1 + 1]
                        o1 = _dc.replace(a_o1, ap=[a_o1.ap[0], [-BFREE, 3], [1, 1]])
                        a_l1 = smm[:, c1 - 1:c1]
                        l1 = _dc.replace(a_l1, ap=[a_l1.ap[0], [BFREE, 3], [1, 1]])
                        a_r1 = msqh(slice(c1, c1 + 1))
                        r1 = _dc.replace(a_r1, ap=[a_r1.ap[0], [0, 3], [1, 1]])
                        nc.vector.tensor_tensor(o1, l1, r1, Op.is_lt)

                    # sel matmuls: vps[pair] += sel @ bits4[plane(pair)][tile]
                    for t in range(TPB):
                        fs = slice(t * W, (t + 1) * W)
                        for ni in range(4):
                            pl = PLANE_OF_PAIR[ni]
                            nc.tensor.matmul(vps[ni][:], mat("sel", h, t),
                                             plane(bits4, pl, fs),
                                             start=(t == 0),
                                             stop=(t == TPB - 1))

                    # band: xk planes, keep chain, mag, out
                    for ni in range(4):
                        nc.vector.tensor_scalar(xk(ni), vps[ni][:], 16.0,
                                                None, Op.is_equal)
                    # e01 = x1-x0, e03 = x3-x0, ee = (x2-x3)-(x1-x0)
                    e01 = ed[:, 0 * W:1 * W]
                    e03 = ed[:, 1 * W:2 * W]
                    ee = ed[:, 2 * W:3 * W]
                    nc.vector.tensor_tensor(e01, xk(1), xk(0), Op.subtract)
                    nc.vector.tensor_tensor(e03, xk(3), xk(0), Op.subtract)
                    nc.vector.tensor_tensor(ee, xk(2), xk(3), Op.subtract)
                    nc.vector.tensor_tensor(ee, ee, e01, Op.subtract)
                    # keep = (x0 + a*e01) + v*(e03 + a*ee)
                    nc.vector.tensor_tensor(keep_b[:].rearrange(
                        "p (s w) -> p s w", w=W), asel_b[:].rearrange(
                        "p (s w) -> p s w", w=W), xrep(e01), Op.mult)
                    nc.vector.tensor_tensor(keep_b[:].rearrange(
                        "p (s w) -> p s w", w=W), keep_b[:].rearrange(
                        "p (s w) -> p s w", w=W), xrep(xk(0)), Op.add)
                    nc.gpsimd.tensor_tensor(t2_b[:].rearrange(
                        "p (s w) -> p s w", w=W), asel_b[:].rearrange(
                        "p (s w) -> p s w", w=W), xrep(ee), Op.mult)
                    nc.gpsimd.tensor_tensor(t2_b[:].rearrange(
                        "p (s w) -> p s w", w=W), t2_b[:].rearrange(
                        "p (s w) -> p s w", w=W), xrep(e03), Op.add)
                    nc.vector.tensor_tensor(t2_b[:], vsel_b[:], t2_b[:],
                                            Op.mult)
                    nc.vector.tensor_tensor(keep_b[:], keep_b[:], t2_b[:],
                                            Op.add)
                    nc.scalar.activation(mag_b[:], msqh(), AF.Sqrt, scale=sc)
                    nc.vector.tensor_tensor(out_b[:], mag_b[:], keep_b[:],
                                            Op.mult)
                    nc.sync.dma_start(out=outp[:, h * BFREE:(h + 1) * BFREE],
                                      in_=out_b[:])

    nc.compile()
    return nc


_NC_CACHE = None


def kernel(img):
    global _NC_CACHE
    img = np.ascontiguousarray(np.asarray(img, dtype=np.float32))
    assert img.shape == (B, C, H, W)

    if _NC_CACHE is None:
        _NC_CACHE = _build_bass()
    nc = _NC_CACHE

    in_maps = [_build_core_inputs(img, core) for core in range(NCORES)]
    trace = bool(os.environ.get("CANNY_TRACE"))
    res = run_bass_kernel_spmd(nc, in_maps, core_ids=list(range(NCORES)),
                               trace=trace)
    if trace and res.exec_time_ns is not None:
        print(f"HW exec time: {res.exec_time_ns} ns")
        kernel.last_exec_ns = res.exec_time_ns

    out = np.zeros((B, C, H, W), np.float32)
    for core in range(NCORES):
        o = np.asarray(res.results[core]["outp"], np.float32)
        for h in range(NBANDS):
            r0b = ROWS * core + SH * h
            for t in range(TPB):
                T = TPB * h + t
                for j, s in enumerate(_slots(t)):
                    blk = o[SIN * j + HALO:SIN * j + HALO + SH,
                            T * W:(T + 1) * W]
                    out[s, :, r0b:r0b + SH, :] = blk[None]
    return out


if __name__ == "__main__":
    img = np.load("/tmp/img.npy")
    out = kernel(img)
    exp = np.load("/tmp/expected.npy")
    d = np.abs(out - exp)
    print("absmax", d.max(), "n>1e-2", (d > 1e-2).sum(),
          "keepmis", ((out != 0) != (exp != 0)).sum())
